# revision 10
# baseline (speedup 1.0000x reference)
"""Transformer block (B=4,T=2048,C=1024,H=16) on 8 trn2 cores, zero-communication.

v3: fully transposed layout (see v2 docstring) with:
  - softmax normalize: evacuate av PSUM early (bf16 casts), paired-head
    [128,512] reciprocal, single fused scale TT per (hp, m)
  - LN rows: broadcast sd row via PE, reciprocal on [128,512] (128 lanes)
  - m-outer attention; c_proj + LN2 + mT for macro m pipelined into the
    next macro's (ACT-bound) attention window
Token order per core is permuted to [own | other]; host stages x/xq
pre-transposed and un-transposes the output.
"""
import sys

sys.path.insert(0, "/opt/trn_rl_repo")

import numpy as np
import ml_dtypes

import concourse.bass as bass
import concourse.mybir as mybir
import concourse.tile as tile
from concourse import bacc

FP32 = mybir.dt.float32
BF16 = mybir.dt.bfloat16
AF = mybir.ActivationFunctionType
ALU = mybir.AluOpType

C = 1024
H = 16
HS = 64
FF = 4096
LN_EPS = 1e-5
P = 128


def build_nc(T=2048):
    own = T // 2
    NCT = C // P          # 8 feature tiles
    NKV = T // P          # 16 kv token tiles
    QM = 512              # q macro width
    NM = own // QM        # 2
    TW = T // 512         # 4 context 512-windows
    NQW = own // 512      # 2 own 512-windows

    nc = bacc.Bacc(None, target_bir_lowering=False, debug=False)

    xT = nc.dram_tensor("xT", [C, T], BF16, kind="ExternalInput")
    xqT = nc.dram_tensor("xqT", [C, own], BF16, kind="ExternalInput")
    wq = nc.dram_tensor("wq", [C, C], BF16, kind="ExternalInput")
    wk = nc.dram_tensor("wk", [C, C], BF16, kind="ExternalInput")
    wv = nc.dram_tensor("wv", [C, C], BF16, kind="ExternalInput")
    wc = nc.dram_tensor("wc", [C, C], BF16, kind="ExternalInput")
    wf = nc.dram_tensor("wf", [C, FF], BF16, kind="ExternalInput")
    wp = nc.dram_tensor("wp", [FF, C], BF16, kind="ExternalInput")
    msk = nc.dram_tensor("msk", [P, 2 * 2 * P], BF16, kind="ExternalInput")
    youtT = nc.dram_tensor("youtT", [C, own], FP32, kind="ExternalOutput")

    with tile.TileContext(nc) as tc:
        import contextlib

        with contextlib.ExitStack() as ctx:
            const = ctx.enter_context(tc.tile_pool(name="const", bufs=1))
            xtp = ctx.enter_context(tc.tile_pool(name="xtp", bufs=9))
            ntp = ctx.enter_context(tc.tile_pool(name="ntp", bufs=9))
            sqp = ctx.enter_context(tc.tile_pool(name="sqp", bufs=3))
            rowp = ctx.enter_context(tc.tile_pool(name="rowp", bufs=3))
            rowbp = ctx.enter_context(tc.tile_pool(name="rowbp", bufs=6))
            bcp_s = ctx.enter_context(tc.tile_pool(name="bcp_s", bufs=2))
            avsb = ctx.enter_context(tc.tile_pool(name="avsb", bufs=4))
            recf = ctx.enter_context(tc.tile_pool(name="recf", bufs=3))
            # 4KB/part: kT -> wf halves -> wp halves
            g1 = ctx.enter_context(tc.tile_pool(name="g1", bufs=NCT))
            # 2.08KB/part x16: vA -> hT
            vap = ctx.enter_context(tc.tile_pool(name="vap", bufs=NKV))
            wqp = ctx.enter_context(tc.tile_pool(name="wqp", bufs=NCT))  # wq -> attnT
            wkp = ctx.enter_context(tc.tile_pool(name="wkp", bufs=NCT))  # wk -> wc
            wvp = ctx.enter_context(tc.tile_pool(name="wvp", bufs=NCT))  # wv -> mT
            qtp = ctx.enter_context(tc.tile_pool(name="qtp", bufs=NCT))  # qT -> yo
            x2p = ctx.enter_context(tc.tile_pool(name="x2p", bufs=NCT))  # x2T bf16
            exq = ctx.enter_context(tc.tile_pool(name="exq", bufs=4))

            ps_sc = ctx.enter_context(tc.tile_pool(name="ps_sc", bufs=2, space="PSUM"))
            ps_av = ctx.enter_context(tc.tile_pool(name="ps_av", bufs=4, space="PSUM"))

            ones_col = const.tile([P, 1], BF16)
            nc.vector.memset(ones_col, 1.0 / C)
            ones1 = const.tile([1, 64], BF16)
            nc.vector.memset(ones1, 1.0)
            ones_row = const.tile([1, P], BF16)
            nc.vector.memset(ones_row, 1.0)
            epst = const.tile([1, 1], FP32)
            nc.vector.memset(epst, LN_EPS)
            warm = const.tile([P, P], BF16)
            nc.vector.memset(warm, 0.001)
            maskt = const.tile([P, 4 * P], BF16)
            nc.scalar.dma_start(maskt[:], msk[:])
            mask4 = maskt.rearrange("p (x r c) -> p x r c", x=2, r=2)

            for wi in range(44):
                wps = ps_av.tile([P, P], FP32, tag="av", name=f"warm{wi}")
                nc.tensor.matmul(wps[:], warm[:], warm[:], start=True, stop=True)

            # first 512-token window of xT lands before the 6MB of weights
            xt0 = []
            for ci in range(NCT):
                xt = xtp.tile([P, 512], BF16, tag="xt", name=f"x0_{ci}")
                eng = nc.sync if ci % 2 == 0 else nc.scalar
                eng.dma_start(xt[:], xT[P * ci : P * (ci + 1), 0:512])
                xt0.append(xt)
            wq_sb, wk_sb, wv_sb = [], [], []
            for ci in range(NCT):
                w = wkp.tile([P, C], BF16, tag="wb", name=f"wk{ci}")
                nc.scalar.dma_start(w[:], wk[P * ci : P * (ci + 1), :])
                wk_sb.append(w)
            for ci in range(NCT):
                w = wqp.tile([P, C], BF16, tag="wa", name=f"wq{ci}")
                nc.scalar.dma_start(w[:], wq[P * ci : P * (ci + 1), :])
                wq_sb.append(w)
                w = wvp.tile([P, C], BF16, tag="wc", name=f"wv{ci}")
                nc.scalar.dma_start(w[:], wv[P * ci : P * (ci + 1), :])
                wv_sb.append(w)

            kT = [g1.tile([P, T], BF16, tag="big", name=f"kT{i}") for i in range(NCT)]
            vA = []
            for tt in range(NKV):
                v = vap.tile([P, H * 65], BF16, tag="va", name=f"vA{tt}")
                v3 = v.rearrange("p (h k) -> p h k", k=65)
                nc.vector.memset(v3[:, :, 64:65], 1.0)
                vA.append(v)
            qT = [qtp.tile([P, own], BF16, tag="qt", name=f"qT{i}") for i in range(NCT)]

            def ln_rows(xtiles, uid, n_feat_tiles):
                """Column-wise LN stats over 512 tokens -> (mub, sdb) bf16 rows."""
                mps = ps_av.tile([1, 512], FP32, tag="av", name=f"mps{uid}")
                sps = ps_av.tile([1, 512], FP32, tag="av", name=f"sps{uid}")
                for ci in range(n_feat_tiles):
                    sq = sqp.tile([P, 512], BF16, tag="sq", name=f"sq{uid}_{ci}")
                    nc.vector.tensor_mul(sq[:], xtiles[ci], xtiles[ci])
                    nc.tensor.matmul(
                        mps[:], ones_col[:], xtiles[ci],
                        start=(ci == 0), stop=(ci == n_feat_tiles - 1),
                    )
                    nc.tensor.matmul(
                        sps[:], ones_col[:], sq[:],
                        start=(ci == 0), stop=(ci == n_feat_tiles - 1),
                    )
                mub = rowbp.tile([1, 512], BF16, tag="rowb", name=f"mub{uid}")
                with nc.allow_low_precision(reason="ln mu row to bf16"):
                    nc.vector.tensor_copy(mub[:], mps[:])
                musq = rowbp.tile([1, 512], BF16, tag="rowb", name=f"musq{uid}")
                nc.vector.tensor_mul(musq[:], mub[:], mub[:])
                vr = rowp.tile([1, 512], FP32, tag="row", name=f"vr{uid}")
                nc.vector.tensor_sub(vr[:], sps[:], musq[:])
                sdb = rowbp.tile([1, 512], BF16, tag="rowb", name=f"sdb{uid}")
                nc.scalar.activation(sdb[:], vr[:], AF.Sqrt, bias=epst[:])
                return mub, sdb

            def ln_bc(mub, sdb, uid):
                """Rows -> [128,512] bf16 bcmu and bcrs (=1/sd) tiles."""
                bps = ps_av.tile([P, 512], FP32, tag="av", name=f"bm{uid}")
                nc.tensor.matmul(bps[:], ones_row[:], mub[:], start=True, stop=True)
                bcmu = bcp_s.tile([P, 512], BF16, tag="bc", name=f"bcm{uid}")
                nc.vector.tensor_copy(bcmu[:], bps[:])
                bps2 = ps_av.tile([P, 512], FP32, tag="av", name=f"bs{uid}")
                nc.tensor.matmul(bps2[:], ones_row[:], sdb[:], start=True, stop=True)
                bcrs = recf.tile([P, 512], FP32, tag="rec", name=f"bcr{uid}")
                nc.vector.reciprocal_approx_fast(out=bcrs[:], in_=bps2[:])
                return bcmu, bcrs

            # ---- Build: LN1 + kT/vA/qT per 512-token window ---------------
            _sc = nc.enter_named_scope("ph_build", False)[0]
            for tw in range(TW):
                if tw == 0:
                    xts = xt0
                else:
                    xts = []
                    for ci in range(NCT):
                        xt = xtp.tile([P, 512], BF16, tag="xt", name=f"x{tw}_{ci}")
                        nc.sync.dma_start(
                            xt[:],
                            xT[P * ci : P * (ci + 1), 512 * tw : 512 * (tw + 1)],
                        )
                        xts.append(xt)
                mub, sdb = ln_rows([x[:] for x in xts], f"a{tw}", NCT)
                bcmu, bcrs = ln_bc(mub, sdb, f"a{tw}")
                nts = []
                for ci in range(NCT):
                    tmp = sqp.tile([P, 512], BF16, tag="sq", name=f"tm{tw}_{ci}")
                    nc.vector.tensor_sub(tmp[:], xts[ci][:], bcmu[:])
                    nt = ntp.tile([P, 512], BF16, tag="nt", name=f"nt{tw}_{ci}")
                    nc.vector.tensor_mul(nt[:], tmp[:], bcrs[:])
                    nts.append(nt)
                for ot in range(NCT):
                    ps = ps_av.tile([P, 512], FP32, tag="av", name=f"kp{tw}_{ot}")
                    for ci in range(NCT):
                        nc.tensor.matmul(
                            ps[:], wk_sb[ci][:, P * ot : P * (ot + 1)], nts[ci][:],
                            start=(ci == 0), stop=(ci == NCT - 1),
                        )
                    nc.vector.tensor_copy(kT[ot][:, 512 * tw : 512 * (tw + 1)], ps[:])
                for u in range(4):
                    tt = 4 * tw + u
                    for oj in range(2):
                        ps = ps_av.tile([P, 512], FP32, tag="av", name=f"vp{tt}_{oj}")
                        for ci in range(NCT):
                            nc.tensor.matmul(
                                ps[:],
                                nts[ci][:, P * u : P * (u + 1)],
                                wv_sb[ci][:, 512 * oj : 512 * (oj + 1)],
                                start=(ci == 0), stop=(ci == NCT - 1),
                            )
                        v3 = vA[tt].rearrange("p (h k) -> p h k", k=65)
                        ps3 = ps.rearrange("p (h k) -> p h k", k=64)
                        nc.vector.tensor_copy(
                            v3[:, 8 * oj : 8 * (oj + 1), 0:64], ps3[:]
                        )
                if tw < NQW:
                    for ot in range(NCT):
                        ps = ps_av.tile([P, 512], FP32, tag="av", name=f"qp{tw}_{ot}")
                        for ci in range(NCT):
                            nc.tensor.matmul(
                                ps[:], wq_sb[ci][:, P * ot : P * (ot + 1)], nts[ci][:],
                                start=(ci == 0), stop=(ci == NCT - 1),
                            )
                        nc.vector.tensor_copy(
                            qT[ot][:, 512 * tw : 512 * (tw + 1)], ps[:]
                        )
            nc.leave_named_scope("ph_build", _sc, False)

            # ---- Attention (m-outer) with pipelined c_proj/LN2 ------------
            _sc = nc.enter_named_scope("ph_attn", False)[0]
            attnT = [
                wqp.tile([P, own], BF16, tag="wa", name=f"attnT{i}")
                for i in range(NCT)
            ]
            wc_sb = []
            for ci in range(NCT):
                w = wkp.tile([P, C], BF16, tag="wb", name=f"wc{ci}")
                nc.scalar.dma_start(w[:], wc[P * ci : P * (ci + 1), :])
                wc_sb.append(w)
            # xq residual pieces into the freed xt slots
            xq_sb = {}
            for ct in range(NCT):
                for twn in range(NQW):
                    t = xtp.tile([P, 512], BF16, tag="xt", name=f"xq{ct}_{twn}")
                    nc.sync.dma_start(
                        t[:], xqT[P * ct : P * (ct + 1), 512 * twn : 512 * (twn + 1)]
                    )
                    xq_sb[(ct, twn)] = t

            x2T = [
                x2p.tile([P, own], BF16, tag="x2", name=f"x2_{i}") for i in range(NCT)
            ]
            mT = [wvp.tile([P, own], BF16, tag="wc", name=f"mT{i}") for i in range(NCT)]

            def attn_macro(m):
                for hp in range(H // 2):
                    items = [(x, j) for x in range(2) for j in range(4 * (m + 1))]
                    avp = [
                        ps_av.tile([65, QM], FP32, tag="av", name=f"av{hp}_{m}_{r}")
                        for r in range(2)
                    ]
                    exs = {}

                    def emit_av(k, r):
                        X, j = items[k]
                        ex, w0 = exs[k]
                        nc.tensor.matmul(
                            avp[r][:, w0:QM],
                            vA[8 * X + j][
                                :, 65 * (2 * hp + r) : 65 * (2 * hp + r) + 65
                            ],
                            ex[:, QM * r + w0 : QM * (r + 1)],
                            start=(k == 0), stop=(k == len(items) - 1),
                        )
                        if r == 1:
                            del exs[k]

                    for k, (X, j) in enumerate(items):
                        w0 = max(0, P * j - QM * m)
                        sc = ps_sc.tile(
                            [P, 2 * QM], FP32, tag="sc", name=f"sc{hp}_{m}_{k}"
                        )
                        for r in range(2):
                            nc.tensor.matmul(
                                sc[:, QM * r + w0 : QM * (r + 1)],
                                kT[hp][
                                    64 * r : 64 * r + 64,
                                    T // 2 * X + P * j : T // 2 * X + P * (j + 1),
                                ],
                                qT[hp][
                                    64 * r : 64 * r + 64, QM * m + w0 : QM * (m + 1)
                                ],
                                start=True, stop=True,
                            )
                        ex = exq.tile(
                            [P, 2 * QM], BF16, tag="ex", name=f"ex{hp}_{m}_{k}"
                        )
                        sc3 = sc.rearrange("p (r q) -> p r q", r=2)
                        ex3 = ex.rearrange("p (r q) -> p r q", r=2)
                        nc.scalar.activation(
                            ex3[:, :, w0:QM], sc3[:, :, w0:QM], AF.Exp
                        )
                        if P * j >= QM * m:
                            nc.vector.tensor_mul(
                                ex3[:, :, w0 : w0 + P],
                                ex3[:, :, w0 : w0 + P],
                                mask4[:, X, :, :],
                            )
                        exs[k] = (ex, w0)
                        if k >= 2:
                            emit_av(k - 2, 0)
                            emit_av(k - 2, 1)
                    for kk in (len(items) - 2, len(items) - 1):
                        if kk >= 0 and kk in exs:
                            emit_av(kk, 0)
                            emit_av(kk, 1)
                    # normalize: evacuate avp fast, 128-lane reciprocal
                    av2 = avsb.tile([P, 512], BF16, tag="avs", name=f"a2{hp}_{m}")
                    srows = []
                    for r in range(2):
                        with nc.allow_low_precision(reason="av to bf16"):
                            nc.vector.tensor_copy(
                                av2[64 * r : 64 * r + 64, :], avp[r][0:64, :]
                            )
                        sr = rowbp.tile([1, QM], BF16, tag="rowb", name=f"sr{hp}_{m}_{r}")
                        with nc.allow_low_precision(reason="softmax sum to bf16"):
                            nc.vector.tensor_copy(sr[:], avp[r][64:65, :])
                        srows.append(sr)
                    bcp = ps_av.tile([P, QM], FP32, tag="av", name=f"bp{hp}_{m}")
                    for r in range(2):
                        nc.tensor.matmul(
                            bcp[64 * r : 64 * r + 64, :], ones1[:], srows[r][:],
                            start=True, stop=True,
                        )
                    bcs = recf.tile([P, QM], FP32, tag="rec", name=f"bb{hp}_{m}")
                    nc.vector.reciprocal_approx_fast(out=bcs[:], in_=bcp[:])
                    nc.vector.tensor_mul(
                        attnT[hp][:, QM * m : QM * (m + 1)], av2[:], bcs[:]
                    )

            def cproj_ln2(twn):
                xsl = []
                for ct in range(NCT):
                    ps = ps_av.tile([P, 512], FP32, tag="av", name=f"cp{ct}_{twn}")
                    for ci in range(NCT):
                        nc.tensor.matmul(
                            ps[:],
                            wc_sb[ci][:, P * ct : P * (ct + 1)],
                            attnT[ci][:, 512 * twn : 512 * (twn + 1)],
                            start=(ci == 0), stop=(ci == NCT - 1),
                        )
                    nc.vector.tensor_add(
                        x2T[ct][:, 512 * twn : 512 * (twn + 1)],
                        ps[:],
                        xq_sb[(ct, twn)][:],
                    )
                    xsl.append(x2T[ct][:, 512 * twn : 512 * (twn + 1)])
                mub, sdb = ln_rows(xsl, f"l{twn}", NCT)
                bcmu, bcrs = ln_bc(mub, sdb, f"l{twn}")
                for ci in range(NCT):
                    tmp = sqp.tile([P, 512], BF16, tag="sq", name=f"lt{twn}_{ci}")
                    nc.vector.tensor_sub(tmp[:], xsl[ci], bcmu[:])
                    nc.vector.tensor_mul(
                        mT[ci][:, 512 * twn : 512 * (twn + 1)], tmp[:], bcrs[:]
                    )

            for m in range(NM):
                attn_macro(m)
                cproj_ln2(m)
            nc.leave_named_scope("ph_attn", _sc, False)

            # ---- MLP ------------------------------------------------------
            _sc = nc.enter_named_scope("ph_mlp", False)[0]
            for fh in range(2):
                wf_sb = []
                for ci in range(NCT):
                    w = g1.tile([P, 2048], BF16, tag="big", name=f"wf{fh}_{ci}")
                    nc.sync.dma_start(
                        w[:], wf[P * ci : P * (ci + 1), 2048 * fh : 2048 * (fh + 1)]
                    )
                    wf_sb.append(w)
                hT = [
                    vap.tile([P, own], BF16, tag="va", name=f"hT{fh}_{i}")
                    for i in range(16)
                ]
                for ftl in range(16):
                    for mq in range(NQW):
                        ps = ps_av.tile(
                            [P, 512], FP32, tag="av", name=f"fp{fh}_{ftl}_{mq}"
                        )
                        for ci in range(NCT):
                            nc.tensor.matmul(
                                ps[:],
                                wf_sb[ci][:, P * ftl : P * (ftl + 1)],
                                mT[ci][:, 512 * mq : 512 * (mq + 1)],
                                start=(ci == 0), stop=(ci == NCT - 1),
                            )
                        nc.scalar.activation(
                            hT[ftl][:, 512 * mq : 512 * (mq + 1)], ps[:],
                            AF.Gelu_apprx_tanh,
                        )
                wp_sb = []
                for ftl in range(16):
                    pool, tg = (qtp, "qt") if ftl < 8 else (g1, "big")
                    w = pool.tile([P, C], BF16, tag=tg, name=f"wp{fh}_{ftl}")
                    r0 = 2048 * fh + P * ftl
                    nc.sync.dma_start(w[:], wp[r0 : r0 + P, :])
                    wp_sb.append(w)
                for ct in range(NCT):
                    for twn in range(NQW):
                        ps = ps_av.tile(
                            [P, 512], FP32, tag="av", name=f"pp{fh}_{ct}_{twn}"
                        )
                        for ftl in range(16):
                            nc.tensor.matmul(
                                ps[:],
                                wp_sb[ftl][:, P * ct : P * (ct + 1)],
                                hT[ftl][:, 512 * twn : 512 * (twn + 1)],
                                start=(ftl == 0), stop=(ftl == 15),
                            )
                        if fh == 0:
                            nc.vector.tensor_add(
                                x2T[ct][:, 512 * twn : 512 * (twn + 1)],
                                ps[:],
                                x2T[ct][:, 512 * twn : 512 * (twn + 1)],
                            )
                        else:
                            yo = recf.tile([P, 512], FP32, tag="rec", name=f"yo{ct}_{twn}")
                            nc.vector.tensor_add(
                                yo[:], ps[:],
                                x2T[ct][:, 512 * twn : 512 * (twn + 1)],
                            )
                            nc.sync.dma_start(
                                youtT[
                                    P * ct : P * (ct + 1),
                                    512 * twn : 512 * (twn + 1),
                                ],
                                yo[:],
                            )
            nc.leave_named_scope("ph_mlp", _sc, False)

    nc.compile()
    return nc


def stage_inputs(x, c_attn_w, c_proj_w, fc_w, proj_w, ln1_g, ln2_g, T=2048, n_cores=8):
    """Host-side prep: per-core input maps. x: (B, T, C) f32."""
    bf = ml_dtypes.bfloat16
    g1w = c_attn_w * ln1_g[:, None]
    wqh = np.ascontiguousarray((g1w[:, 0:C] * 0.125).astype(bf))
    wkh = np.ascontiguousarray(g1w[:, C : 2 * C].astype(bf))
    wvh = np.ascontiguousarray(g1w[:, 2 * C : 3 * C].astype(bf))
    wch = np.ascontiguousarray(c_proj_w.astype(bf))
    wfh = np.ascontiguousarray((fc_w * ln2_g[:, None]).astype(bf))
    wph = np.ascontiguousarray(proj_w.astype(bf))
    pp = np.arange(P)[:, None]
    cc = np.arange(P)[None, :]
    mA = (pp <= cc).astype(np.float32)
    in_maps = []
    for c in range(n_cores):
        b, s = c // 2, c % 2
        perm = np.r_[np.arange(s, T, 2), np.arange(1 - s, T, 2)]
        xTv = np.ascontiguousarray(x[b][perm].T.astype(bf))
        xqTv = np.ascontiguousarray(x[b][s::2].T.astype(bf))
        mB = ((pp < cc) if s == 0 else (pp <= cc)).astype(np.float32)
        mk = np.stack([mA, mB], axis=1)
        mk = np.repeat(mk[:, :, None, :], 2, axis=2)
        in_maps.append(
            {
                "xT": xTv,
                "xqT": xqTv,
                "wq": wqh,
                "wk": wkh,
                "wv": wvh,
                "wc": wch,
                "wf": wfh,
                "wp": wph,
                "msk": np.ascontiguousarray(mk.reshape(P, 4 * P).astype(bf)),
            }
        )
    return in_maps


_NC_CACHE = {}


def _get_nc(T=2048):
    if T not in _NC_CACHE:
        _NC_CACHE[T] = build_nc(T=T)
    return _NC_CACHE[T]


def kernel(**inputs):
    """Full transformer block on 8 NeuronCores. Takes/returns full numpy arrays."""
    from concourse.bass_utils import run_bass_kernel_spmd

    x = np.asarray(inputs["x"], dtype=np.float32)
    B, T, C_ = x.shape
    nc = _get_nc(T=T)
    in_maps = stage_inputs(
        x,
        np.asarray(inputs["c_attn_w"], dtype=np.float32),
        np.asarray(inputs["c_proj_w"], dtype=np.float32),
        np.asarray(inputs["fc_w"], dtype=np.float32),
        np.asarray(inputs["proj_w"], dtype=np.float32),
        np.asarray(inputs["ln1_g"], dtype=np.float32),
        np.asarray(inputs["ln2_g"], dtype=np.float32),
        T=T,
        n_cores=8,
    )
    res = run_bass_kernel_spmd(nc, in_maps, list(range(8)))
    out = np.empty((B, T, C_), dtype=np.float32)
    for c in range(8):
        b, s = c // 2, c % 2
        out[b, s::2, :] = res.results[c]["youtT"].T
    return out
